# revision 44
# baseline (speedup 1.0000x reference)
"""Depth-weighted 3x3 conv (DepthConv) Trainium2 Bass kernel.

out[b,o,h,w] = sum_{c,i,j} img_pad[b,c,h+i,w+j] * exp(-8.3*|d[b,h,w]-d_pad[b,h+i,w+j]|)
               * weight[o,c,i,j]  + bias[o]

Sharding: data-parallel over batch, one batch element per NeuronCore (8 cores).

Per-core plan (all heavy compute on device):
  - INTERLEAVED partition layout everywhere: partition p = 2*c + h where
    c = input channel (0..63) and h = tap-parity within a pair.  The
    3x3x64 = 576-row contraction is split into 5 K=128 chunks of
    (2 taps x 64 ch); the center tap rides chunk 4 with zero weights on
    odd partitions.
  - Taps are paired so both halves of a 128-partition op share ONE uniform
    free-dim offset: even partitions hold a padded image copy at byte
    offset s, odd at 0, with s = delta_odd - delta_even.
    imgA (s=1):   pairs (0,1) and (7,8);  imgB (s=128): (2,3) and (5,6).
    Both image tiles are loaded straight from HBM (bf16 from host) with
    partition-strided dsts -- no on-chip copies at all.
  - dw = exp(-8.3*|dp - center|) is computed on chip, scrambled to a
    group-major DRAM scratch dwT2[(g,h,b), px] so that ONE DMA per group
    broadcasts all 8 taps x 64 channels with 16KB descriptors and a
    64-wide outer dim (split across all 16 SDMA engines).  Keeping the
    main loop at 2 DMAs per group matters: the tile framework serializes
    DMAs sharing one of its 8 completion-semaphore lanes on ~7us
    completion latency, so DMA COUNT (not just bytes) sets the cadence.
  - z = img_patch * dw built by DVE tensor-tensor multiplies at bf16 2x
    (one [128, 2048] op per pair; all on DVE -- gpsimd tensor ops contend
    for SBUF ports and halve DVE throughput).
  - Matmuls accumulate 5 chunks into 4 PSUM tiles [64 out-ch, 512 px];
    bias is added by the scalar engine which also downcasts to bf16;
    one batched store per group.
"""

import numpy as np


def _setup_path():
    try:
        import concourse.bass  # noqa: F401
    except ImportError:
        import sys

        for p in ("/opt/trn_rl_repo", "/root/.axon_site/_ro/trn_rl_repo"):
            if p not in sys.path:
                sys.path.insert(0, p)


_setup_path()

import ml_dtypes  # noqa: E402
import concourse.bass as bass  # noqa: E402
import concourse.mybir as mybir  # noqa: E402
import concourse.tile as tile  # noqa: E402
from concourse.bass_types import AP  # noqa: E402
from concourse.mybir import (  # noqa: E402
    ActivationFunctionType as ACTF,
)

dt = mybir.dt

B, CIN, COUT, H, W = 8, 64, 64, 128, 128
HW = H * W  # 16384
WP = 130  # padded row length
PADSZ = WP * WP  # 16900
ALPHA = 8.3
N_CORES = 8

# tap t = 3*i + j ; padded-flat offset delta_t = 130*i + j
DELTA = [130 * i + j for i in range(3) for j in range(3)]
# pairs b: (even tap, odd tap); even copy shifted by delta_odd - delta_even
PAIRS = [(0, 1), (7, 8), (2, 3), (5, 6)]
PAIR_IMG = ["A", "A", "B", "B"]
PAIR_BASE = [1 + DELTA[0], 1 + DELTA[7], 128 + DELTA[2], 128 + DELTA[5]]
CENTER_BASE = 1 + DELTA[4]  # even lanes see the center tap; odd get zero wt
# dw slot s' in dw9 free dim, h-major: s' = h*4 + b -> tap
SLOT_TAPS = [0, 7, 2, 5, 1, 8, 3, 6]

GROUPS = 8
GPX = HW // GROUPS  # 2048 px per group (16 image rows)
TILES = 4  # matmul tiles per group
TPX = GPX // TILES  # 512 px per tile (matmul N max on this encoding)

IMGA_LEN = 1 + PADSZ + 8
IMGB_LEN = 128 + PADSZ + 8


def _win(ap_tile, base, nrows):
    """[128, nrows, 128] window AP over a padded flat image tile."""
    v = ap_tile[0:128, base : base + nrows * WP]
    return v.rearrange("p (r w) -> p r w", r=nrows)[:, :, 0:W]


def _body(tc, img_d, dep_d, wt_d, bias_d, out_d, reps=1):
    nc = tc.nc
    f32, bf16 = dt.float32, dt.bfloat16

    out_flat = out_d.rearrange("o h w -> o (h w)")

    with (
        tc.tile_pool(name="big", bufs=1) as big,
        tc.tile_pool(name="consts", bufs=1) as consts,
        tc.tile_pool(name="dw", bufs=1) as dwp,
        tc.tile_pool(name="dwb", bufs=3) as dwbp,
        tc.tile_pool(name="z", bufs=2) as zp,
        tc.tile_pool(name="osb", bufs=2) as osb,
        tc.tile_pool(name="psum", bufs=2, space="PSUM") as psp,
    ):
        # ---- depth windows first (heads the dw chain) -------------------
        Dall = consts.tile([128, 3 * WP], f32, tag="Dall")
        dep_src = AP(
            tensor=dep_d.tensor,
            offset=0,
            ap=[[WP, 128], [WP, 3], [1, WP]],
        )
        nc.sync.dma_start(
            Dall[:, :].rearrange("p (i jw) -> p i jw", i=3), dep_src
        )

        # ---- dw = exp(-8.3*|dp - center|), layout [h, s'*128 + w] -------
        dwd = dwp.tile([128, 8 * W], f32, tag="dwd")
        center = Dall[:, WP + 1 : WP + 1 + W]
        for s, t in enumerate(SLOT_TAPS):
            i, j = divmod(t, 3)
            nc.vector.tensor_sub(
                dwd[:, s * W : (s + 1) * W],
                Dall[:, i * WP + j : i * WP + j + W],
                center,
            )
        dw9 = dwp.tile([128, 8 * W], bf16, tag="dw9")
        HW4 = 4 * W
        for h in range(2):
            sl = slice(h * HW4, (h + 1) * HW4)
            nc.scalar.activation(dwd[:, sl], dwd[:, sl], ACTF.Abs, scale=-ALPHA)
            nc.scalar.activation(dw9[:, sl], dwd[:, sl], ACTF.Exp, scale=-1.0)

        # scramble to group-major DRAM scratch:
        # dwT2[(g*2+h)*4 + b, px] = dw of (pair b, parity h) at group-g px
        dwT2_d = nc.dram_tensor(
            "dwT2_scratch", (64, GPX), dt.bfloat16, kind="Internal"
        ).ap()
        for s in range(8):  # s = h*4 + b
            h, b = divmod(s, 4)
            ddst = AP(
                tensor=dwT2_d.tensor,
                offset=(h * 4 + b) * GPX,
                ap=[[8 * GPX, 8], [W, 16], [1, W]],
            )
            # gpsimd (SWDGE) queue: otherwise idle, so these 8 writes never
            # sit behind image loads or block the dwb stream on sync.
            nc.gpsimd.dma_start(ddst, dw9[:, s * W : (s + 1) * W])

        # ---- image loads AFTER the dwT writes in emission order ---------
        # DMA-completion sem lanes are handed out round-robin by emission:
        # the tiny dwT writes must grab lanes BEFORE the 2.2MB image loads,
        # or each write chains behind an image-load completion (~7us each).
        # Host ships the tiles pre-interleaved (partition p = 2c + h, even
        # copy shifted), so these are plain contiguous loads on scalar.
        imgA = big.tile([128, IMGA_LEN], bf16, tag="imgA")
        imgB = big.tile([128, IMGB_LEN], bf16, tag="imgB")
        imgA_d, imgB_d = img_d
        for dst, src, ln in (
            (imgA, imgA_d, IMGA_LEN),
            (imgB, imgB_d, IMGB_LEN),
        ):
            half = ln // 2
            nc.scalar.dma_start(dst[:, 0:half], src[:, 0:half])
            nc.scalar.dma_start(dst[:, half:ln], src[:, half:ln])

        # ---- constants (host-packed, one DMA) ---------------------------
        wc = consts.tile([128, 5 * 64], bf16, tag="wc")
        nc.scalar.dma_start(wc[:, :], wt_d[:, :])
        bias_t = consts.tile([64, 1], f32, tag="bias")
        nc.scalar.dma_start(bias_t[:, :], bias_d)

        # ---- main loop --------------------------------------------------
        img_tiles = {"A": imgA, "B": imgB}

        for gi in range(GROUPS * reps):
            g = gi % GROUPS
            r0 = (g * GPX) // W
            # ONE dw broadcast DMA per group on sync (only dwb rides sync,
            # so the stream is never head-of-line blocked): src outer dim =
            # 64 replicas (16-engine split), inner run = 16KB.
            dwb = dwbp.tile([128, 4 * GPX], bf16, tag="dwb", name="dwb")
            dsrc = AP(
                tensor=dwT2_d.tensor,
                offset=g * 8 * GPX,
                ap=[[0, 64], [4 * GPX, 2], [1, 4 * GPX]],
            )
            nc.sync.dma_start(dwb[:, :], dsrc)
            # z build: one DVE op per pair-block [128, 2048]
            zs = []
            for p in range(4):
                z = zp.tile([128, GPX], bf16, tag=f"z{p}", name=f"z{p}")
                nc.vector.tensor_mul(
                    z[:, :].rearrange("p (r w) -> p r w", w=W),
                    _win(img_tiles[PAIR_IMG[p]], PAIR_BASE[p] + r0 * WP, 16),
                    dwb[:, p * GPX : (p + 1) * GPX].rearrange(
                        "p (r w) -> p r w", w=W
                    ),
                )
                zs.append(z)
            # matmuls: chunk-outer over 4 PSUM tiles (center chunk starts)
            pss = [
                psp.tile([64, TPX], f32, tag=f"ps{it}", name=f"ps{it}")
                for it in range(TILES)
            ]
            for it in range(TILES):
                nc.tensor.matmul(
                    pss[it][:, :],
                    wc[:, 4 * 64 : 5 * 64],
                    _win(imgA, CENTER_BASE + (r0 + 4 * it) * WP, 4),
                    start=True,
                    stop=False,
                )
            for p in range(4):
                for it in range(TILES):
                    nc.tensor.matmul(
                        pss[it][:, :],
                        wc[:, p * 64 : (p + 1) * 64],
                        zs[p][:, it * TPX : (it + 1) * TPX].rearrange(
                            "p (r w) -> p r w", w=W
                        ),
                        start=False,
                        stop=(p == 3),
                    )
            # bias + downcast + two half-group stores (the second half can
            # drain while the next group computes; trims the kernel tail)
            ob = osb.tile([64, GPX], bf16, tag="ob", name="ob")
            for it in range(TILES):
                nc.scalar.activation(
                    ob[:, it * TPX : (it + 1) * TPX],
                    pss[it][:, :],
                    ACTF.Identity,
                    bias=bias_t[:, 0:1],
                )
                if it % 2 == 1:
                    lo = (it - 1) * TPX
                    nc.scalar.dma_start(
                        out_flat[:, g * GPX + lo : g * GPX + lo + 2 * TPX],
                        ob[:, lo : lo + 2 * TPX],
                    )


def _split_multiwaits(nc):
    """TRN2 codegen allows a single sync-wait per instruction; Tile can emit
    more at multi-producer joins.  Move surplus waits onto standalone
    EventSemaphore instructions just before the instruction, same engine."""
    n = 0
    for fn in nc.m.functions:
        for blk in fn.blocks:
            idx = 0
            while idx < len(blk.instructions):
                inst = blk.instructions[idx]
                si = inst.sync_info
                if si is not None and len(si.on_wait) > 1:
                    waits = list(si.on_wait)
                    for w in waits[:-1]:
                        ev = mybir.InstEventSemaphore(
                            name=f"wsplit-{nc.next_id()}",
                            ins=[],
                            outs=[],
                            sync_info=mybir.SyncInfo(on_wait=[w], on_update=[]),
                        )
                        ev.engine = inst.engine
                        nc.register_instruction(ev)
                        blk.instructions.insert(idx, ev)
                        idx += 1
                        n += 1
                    inst.sync_info = mybir.SyncInfo(
                        on_wait=[waits[-1]], on_update=list(si.on_update)
                    )
                idx += 1
    return n


_CACHE = {}


def _build(reps=1):
    key = ("nc", reps)
    if key not in _CACHE:
        nc = bass.Bass(
            "TRN2", target_bir_lowering=False, debug=False, num_devices=N_CORES
        )
        imgA_d = nc.dram_tensor("imgA", (128, IMGA_LEN), dt.bfloat16, kind="ExternalInput").ap()
        imgB_d = nc.dram_tensor("imgB", (128, IMGB_LEN), dt.bfloat16, kind="ExternalInput").ap()
        img_d = (imgA_d, imgB_d)
        dep_d = nc.dram_tensor("depth", (WP, WP), dt.float32, kind="ExternalInput").ap()
        wt_d = nc.dram_tensor("wt", (128, 5 * 64), dt.bfloat16, kind="ExternalInput").ap()
        bias_d = nc.dram_tensor("bias", (64, 1), dt.float32, kind="ExternalInput").ap()
        out_d = nc.dram_tensor("out", (COUT, H, W), dt.bfloat16, kind="ExternalOutput").ap()
        with tile.TileContext(nc) as tc:
            _body(tc, img_d, dep_d, wt_d, bias_d, out_d, reps=reps)
        _split_multiwaits(nc)
        _CACHE[key] = nc
    return _CACHE[key]


def _host_weights(weight):
    w = np.asarray(weight, dtype=np.float32)  # [o, c, i, j]
    wt = np.zeros((5, 128, 64), dtype=np.float32)
    for k, (ta, tb) in enumerate(PAIRS):
        for c in range(64):
            wt[k, 2 * c, :] = w[:, c, ta // 3, ta % 3]
            wt[k, 2 * c + 1, :] = w[:, c, tb // 3, tb % 3]
    for c in range(64):
        wt[4, 2 * c, :] = w[:, c, 1, 1]  # center on even lanes; odd stay 0
    # pack per-partition: wc[p, k*64+o] so the device loads it in ONE DMA
    return np.ascontiguousarray(wt.transpose(1, 0, 2).reshape(128, 5 * 64)).astype(
        ml_dtypes.bfloat16
    )


def make_in_maps(img, depth, weight, bias):
    wt = _host_weights(weight)
    bias_h = np.asarray(bias, dtype=np.float32).reshape(COUT, 1)
    img = np.asarray(img, dtype=np.float32)
    depth = np.asarray(depth, dtype=np.float32)
    img_p = (
        np.pad(img, ((0, 0), (0, 0), (1, 1), (1, 1)))
        .reshape(B, CIN, PADSZ)
        .astype(ml_dtypes.bfloat16)
    )
    # pre-interleaved image tiles: partition p = 2c + h; even copy shifted
    # by +1 (imgA) / +128 (imgB) elements, odd copy at 0.
    imgA_h = np.zeros((B, 128, IMGA_LEN), dtype=ml_dtypes.bfloat16)
    imgB_h = np.zeros((B, 128, IMGB_LEN), dtype=ml_dtypes.bfloat16)
    imgA_h[:, 0::2, 1 : 1 + PADSZ] = img_p
    imgA_h[:, 1::2, 0:PADSZ] = img_p
    imgB_h[:, 0::2, 128 : 128 + PADSZ] = img_p
    imgB_h[:, 1::2, 0:PADSZ] = img_p
    dep_p = np.pad(depth[:, 0], ((0, 0), (1, 1), (1, 1)))
    return [
        {
            "imgA": np.ascontiguousarray(imgA_h[b]),
            "imgB": np.ascontiguousarray(imgB_h[b]),
            "depth": np.ascontiguousarray(dep_p[b]),
            "wt": wt,
            "bias": bias_h,
        }
        for b in range(B)
    ]


def _runner(reps=1):
    """Persistent jitted 8-core executor (compile once per process)."""
    rkey = ("run", reps)
    if rkey in _CACHE:
        return _CACHE[rkey]
    import jax
    from jax.sharding import Mesh, PartitionSpec
    from jax.experimental.shard_map import shard_map
    from concourse.bass2jax import (
        _bass_exec_p,
        install_neuronx_cc_hook,
        partition_id_tensor,
    )

    nc = _build(reps=reps)
    install_neuronx_cc_hook()

    pid_name = nc.partition_id_tensor.name if nc.partition_id_tensor else None
    in_names, out_names, out_avals = [], [], []
    for alloc in nc.m.functions[0].allocations:
        if not isinstance(alloc, mybir.MemoryLocationSet):
            continue
        name = alloc.memorylocations[0].name
        if alloc.kind == "ExternalInput":
            if name != pid_name:
                in_names.append(name)
        elif alloc.kind == "ExternalOutput":
            out_names.append(name)
            out_avals.append(
                jax.core.ShapedArray(
                    tuple(alloc.tensor_shape), mybir.dt.np(alloc.dtype)
                )
            )
    n_params = len(in_names)
    all_in = in_names + out_names  # zero-init output buffers ride as inputs
    if pid_name is not None:
        all_in = all_in + [pid_name]
    donate = tuple(range(n_params, n_params + len(out_names)))

    def _bass_body(*args):
        operands = list(args)
        if pid_name is not None:
            operands.append(partition_id_tensor())
        return tuple(
            _bass_exec_p.bind(
                *operands,
                out_avals=tuple(out_avals),
                in_names=tuple(all_in),
                out_names=tuple(out_names),
                lowering_input_output_aliases=(),
                sim_require_finite=True,
                sim_require_nnan=True,
                nc=nc,
            )
        )

    devices = jax.devices()[:N_CORES]
    mesh = Mesh(np.asarray(devices), ("core",))
    nin = n_params + len(out_names)
    sharded = jax.jit(
        shard_map(
            _bass_body,
            mesh=mesh,
            in_specs=(PartitionSpec("core"),) * nin,
            out_specs=(PartitionSpec("core"),) * len(out_names),
            check_rep=False,
        ),
        donate_argnums=donate,
        keep_unused=True,
    )
    run = (sharded, in_names, out_names, out_avals, mesh)
    _CACHE[rkey] = run
    return run


def _concat_inputs(in_maps, in_names):
    return [
        np.concatenate([np.asarray(m[name]) for m in in_maps], axis=0)
        for name in in_names
    ]


def _zero_outs(out_avals):
    return [
        np.zeros((N_CORES * a.shape[0], *a.shape[1:]), a.dtype) for a in out_avals
    ]


def kernel(img, depth, weight, bias):
    sharded, in_names, out_names, out_avals, _ = _runner()
    in_maps = make_in_maps(img, depth, weight, bias)
    concat_in = _concat_inputs(in_maps, in_names)
    out_arrs = sharded(*concat_in, *_zero_outs(out_avals))
    oi = out_names.index("out")
    out = np.asarray(out_arrs[oi]).reshape(N_CORES, COUT, H, W)
    return out.astype(np.float32)
